# revision 35
# baseline (speedup 1.0000x reference)
"""Trainium2 Bass kernel for a GPT-2 style transformer block (nn_Block_16690242913196).

Sharding (8 NeuronCores, SPMD):
  - Token-parallel spine with batch-interleaved ownership: core c owns
    b0[256c:256c+256] ++ b1[256c:256c+256]  (local cols: [0:256]=batch0,
    [256:512]=batch1).  "half" == batch everywhere.
  - QKV: each core computes q/k/v (all heads) for its own 512 tokens in
    fp8 (DoubleRow matmuls), then a per-batch AllToAll redistributes
    head-slices so core c gets heads {2c,2c+1} for all tokens.
  - Attention: head-parallel, causal-aware, fp8 operands, batched softmax
    reciprocal, denominators via an appended ones-column in V.
  - Per-batch AllToAll returns attention outputs to token-parallel form;
    proj (bf16) + residual; LN2; MLP in fp8 DoubleRow.
  - Collectives are split per batch so they overlap with compute of the
    other batch.  Elementwise work is spread over DVE / Pool / Act.
"""

import numpy as np
import ml_dtypes

P = 128
B, S, D, H = 2, 2048, 1024, 16
DH = D // H          # 64
DI = 4 * D           # 4096
EPS = 1e-5
NCORES = 8
TOK = 512            # tokens per core (256 per batch)
HTOK = 256           # tokens per (core, batch)
KD = D // P          # 8
KDI = DI // P        # 32
QCH = 256            # query chunk == one peer's batch-slice
HL = H // NCORES     # 2 local heads
RG = [list(range(NCORES))]
SWA = 32.0           # host-side scale on attn_w q/k sections (fp8 range use)
FC1_DR = False        # fc1 in fp8 DoubleRow (mT/fw fp8)
SWF = 32.0           # fc_w scale (only when FC1_DR)
MASKB = -1.0e5

_CACHED_NC = None


def build_nc():
    import concourse.bacc as bacc
    import concourse.tile as tile
    import concourse.mybir as mybir
    from contextlib import ExitStack

    dt = mybir.dt
    f32, bf16, f32r, fp8 = dt.float32, dt.bfloat16, dt.float32r, dt.float8e4
    AF = mybir.ActivationFunctionType
    OP = mybir.AluOpType
    DR = mybir.MatmulPerfMode.DoubleRow

    nc = bacc.Bacc("TRN2", target_bir_lowering=False, debug=False,
                   num_devices=NCORES)

    # ---- kernel I/O (per-core shapes) ----
    xT = nc.dram_tensor("xT", [D, TOK], f32r, kind="ExternalInput").ap()
    awqk = nc.dram_tensor("awqk", [P, KD, 2 * D], fp8,
                          kind="ExternalInput").ap()
    awv = nc.dram_tensor("awv", [P, KD, D], bf16, kind="ExternalInput").ap()
    ab = nc.dram_tensor("ab", [3 * D], f32, kind="ExternalInput").ap()
    pw = nc.dram_tensor("pw", [KD, P, KD, P], bf16, kind="ExternalInput").ap()
    pb = nc.dram_tensor("pb", [P, KD], f32, kind="ExternalInput").ap()
    fw = nc.dram_tensor("fw", [KDI, P, KD, P], fp8 if FC1_DR else bf16,
                        kind="ExternalInput").ap()
    fb = nc.dram_tensor("fb", [P, KDI], f32, kind="ExternalInput").ap()
    gw = nc.dram_tensor("gw", [KD, P, KDI, P], bf16, kind="ExternalInput").ap()
    gb = nc.dram_tensor("gb", [P, KD], f32, kind="ExternalInput").ap()
    l1g = nc.dram_tensor("l1g", [P, KD], f32, kind="ExternalInput").ap()
    l1b = nc.dram_tensor("l1b", [P, KD], f32, kind="ExternalInput").ap()
    l2g = nc.dram_tensor("l2g", [P, KD], f32, kind="ExternalInput").ap()
    l2b = nc.dram_tensor("l2b", [P, KD], f32, kind="ExternalInput").ap()
    mk = nc.dram_tensor("mk", [2, P, QCH], f32, kind="ExternalInput").ap()
    sel = nc.dram_tensor("sel", [97, 2, P], bf16,
                         kind="ExternalInput").ap()
    outT = nc.dram_tensor("outT", [D, TOK], f32, kind="ExternalOutput").ap()

    with tile.TileContext(nc) as tc, ExitStack() as ctx:
        const = ctx.enter_context(tc.tile_pool(name="const", bufs=1))
        dram = ctx.enter_context(tc.tile_pool(name="dram", bufs=1, space="DRAM"))
        rows = ctx.enter_context(tc.tile_pool(name="rows", bufs=4))
        lnt = ctx.enter_context(tc.tile_pool(name="lnt", bufs=4))
        sqp = ctx.enter_context(tc.tile_pool(name="sqp", bufs=2))
        res = ctx.enter_context(tc.tile_pool(name="res", bufs=1))
        psum = ctx.enter_context(tc.tile_pool(name="psum", bufs=1, space="PSUM"))
        prs = ctx.enter_context(tc.tile_pool(name="prs", bufs=1))
        wgt = ctx.enter_context(tc.tile_pool(name="wgt", bufs=1))

        # residual spine first in the DMA queue (LN1 is the first consumer)
        xT_sb = res.tile([P, KD, TOK], f32r)
        for hh in range(2):
            for kk in range(0, KD, 2):
                nc.sync.dma_start(
                    xT_sb[:, kk:kk + 2, HTOK * hh:HTOK * (hh + 1)],
                    xT[P * kk:P * (kk + 2),
                       HTOK * hh:HTOK * (hh + 1)].rearrange(
                        "(k p) c -> p k c", p=P))
        h1T = res.tile([P, KD, TOK], f32r)

        # ---- constants ----
        qb_sb = const.tile([P, KD], f32)
        nc.sync.dma_start(qb_sb, ab[0:D].rearrange("(k p) -> p k", p=P))
        kb_sb = const.tile([P, KD], f32)
        nc.sync.dma_start(kb_sb, ab[D:2 * D].rearrange("(k p) -> p k", p=P))
        vbc = const.tile([P, KD, P], bf16)
        nc.gpsimd.dma_start(vbc, ab[2 * D:3 * D].rearrange(
            "(o j c) -> o j c", o=1, j=KD).to_broadcast((P, KD, P)))
        mkb = const.tile([P, 2, QCH], bf16)
        nc.gpsimd.dma_start(mkb, mk.rearrange("m p q -> p m q"))
        l1g_sb = const.tile([P, KD], f32)
        nc.sync.dma_start(l1g_sb, l1g)
        l1b_sb = const.tile([P, KD], f32)
        nc.sync.dma_start(l1b_sb, l1b)
        l2g_sb = const.tile([P, KD], f32)
        nc.sync.dma_start(l2g_sb, l2g)
        l2b_sb = const.tile([P, KD], f32)
        nc.sync.dma_start(l2b_sb, l2b)
        pb_sb = const.tile([P, KD], f32)
        nc.sync.dma_start(pb_sb, pb)
        fb_sb = const.tile([P, KDI], f32)
        nc.sync.dma_start(fb_sb, fb)
        gb_sb = const.tile([P, KD], f32)
        nc.sync.dma_start(gb_sb, gb)
        ones_cf = const.tile([P, 1], f32)
        nc.vector.memset(ones_cf, 1.0)
        ones_c = const.tile([P, 1], f32r)
        nc.vector.tensor_copy(ones_c, ones_cf)
        ones_rf = const.tile([1, P], f32)
        nc.vector.memset(ones_rf, 1.0)
        ones_r = const.tile([1, P], f32r)
        nc.vector.tensor_copy(ones_r, ones_rf)
        sel16 = const.tile([97, 2, P], bf16)
        nc.sync.dma_start(sel16, sel)
        eps_sb = const.tile([1, 1], f32)
        nc.vector.memset(eps_sb, EPS)

        awqk_sb = const.tile([P, KD, 2 * D], fp8)
        nc.sync.dma_start(awqk_sb, awqk)
        awv_sb = const.tile([P, KD, D], bf16)
        nc.sync.dma_start(awv_sb, awv)

        # collective bounce buffers (per batch/half)
        PAYQ = [dram.tile([NCORES, 2, P, QCH], fp8, name=f"cqin{h}")
                for h in range(2)]
        PAYQO = [dram.tile([NCORES, 2, P, QCH], fp8, name=f"cqout{h}")
                 for h in range(2)]
        PAYV = [dram.tile([NCORES, P, QCH], bf16, name=f"cvin{h}")
                for h in range(2)]
        PAYVO = [dram.tile([NCORES, P, QCH], bf16, name=f"cvout{h}")
                 for h in range(2)]
        cc2_in = [dram.tile([NCORES, P, QCH], bf16, name=f"c2in{h}")
                  for h in range(2)]
        cc2_out = [dram.tile([NCORES, P, QCH], bf16, name=f"c2out{h}")
                   for h in range(2)]

        def emit_ln(src_fn, g_sb, b_sb, dst_fn, tag, hh):
            """LayerNorm over the feature (partition) axis for token half hh.
            Stats via f32r ones-matmuls; normalize split across DVE+Pool;
            final scale/shift/cast on Act."""
            c0, c1 = HTOK * hh, HTOK * (hh + 1)
            sx = psum.tile([1, HTOK], f32, tag="big", bufs=2)
            sxx = psum.tile([1, HTOK], f32, tag="big", bufs=2)
            for k in range(KD):
                nc.tensor.matmul(sx, ones_c, src_fn(k)[:, c0:c1],
                                 start=(k == 0), stop=(k == KD - 1),
                                 skip_group_check=True)
            for k in range(KD):
                sq = sqp.tile([P, HTOK], f32r, tag="sq")
                nc.scalar.activation(sq, src_fn(k)[:, c0:c1], AF.Square)
                nc.tensor.matmul(sxx, ones_c, sq,
                                 start=(k == 0), stop=(k == KD - 1),
                                 skip_group_check=True)
            mu = rows.tile([1, HTOK], f32r, tag="row")
            nc.vector.tensor_scalar_mul(mu, sx, 1.0 / D)
            m2 = rows.tile([1, HTOK], f32, tag="row")
            nc.vector.tensor_scalar_mul(m2, sxx, 1.0 / D)
            var = rows.tile([1, HTOK], f32, tag="row")
            nc.vector.tensor_tensor(out=var, in0=mu, in1=mu, op=OP.mult)
            nc.vector.tensor_tensor(out=var, in0=m2, in1=var, op=OP.subtract)
            sd = rows.tile([1, HTOK], f32, tag="row")
            nc.scalar.activation(sd, var, AF.Sqrt, bias=eps_sb[:])
            mub_p = psum.tile([P, HTOK], f32, tag="big", bufs=2)
            nc.tensor.matmul(mub_p, ones_r, mu,
                             start=True, stop=True)
            mub = sqp.tile([P, HTOK], f32, tag="mub", bufs=2)
            nc.vector.tensor_copy(mub, mub_p)
            # subtract pass first (doesn't need rstd); reciprocal overlaps
            t1s = []
            for k in range(KD):
                t1 = lnt.tile([P, HTOK], f32, tag=f"t{k % 2}")
                eng = nc.vector if k % 2 == 0 else nc.gpsimd
                eng.tensor_tensor(out=t1, in0=src_fn(k)[:, c0:c1], in1=mub,
                                  op=OP.subtract)
                t1s.append(t1)
            rstd = rows.tile([1, HTOK], f32r, tag="row")
            with nc.allow_low_precision(reason="f32r rstd for bcast matmul"):
                nc.vector.reciprocal(rstd, sd)
            rsb_p = psum.tile([P, HTOK], f32, tag="big", bufs=2)
            nc.tensor.matmul(rsb_p, ones_r, rstd,
                             start=True, stop=True)
            rsb = sqp.tile([P, HTOK], f32, tag="rsb", bufs=2)
            nc.vector.tensor_copy(rsb, rsb_p)
            for k in range(KD):
                nc.gpsimd.tensor_tensor(out=t1s[k], in0=t1s[k], in1=rsb,
                                        op=OP.mult)
                nc.scalar.activation(dst_fn(k, c0, c1), t1s[k], AF.Identity,
                                     bias=b_sb[:, k:k + 1],
                                     scale=g_sb[:, k:k + 1])

        # ======== pools: mlp + attention-era first (LIFO), then LN1+QKV ==
        mlp2 = ctx.enter_context(tc.tile_pool(name="mlp2", bufs=1))
        mT2 = mlp2.tile([P, KD, TOK], fp8 if FC1_DR else bf16)
        s_att = ExitStack()
        ats = s_att.enter_context(tc.tile_pool(name="ats", bufs=1))
        wp = s_att.enter_context(tc.tile_pool(name="wp", bufs=2))
        s_qkv = ExitStack()
        qkvs = s_qkv.enter_context(tc.tile_pool(name="qkvs", bufs=1))

        xnT = qkvs.tile([P, KD, TOK], bf16)
        xn8 = qkvs.tile([P, KD, TOK], fp8)

        def qkv_half(hh):
            c0, c1 = HTOK * hh, HTOK * (hh + 1)
            qkl = qkvs.tile([P, 2, KD, HTOK], fp8, tag="qkl", bufs=1)
            vl = qkvs.tile([P, 2, KD, P], bf16, tag="vl", bufs=1)
            qkloc = {hh: qkl}
            vloc = {hh: vl}
            with nc.named_scope("qkv"):
                for which in range(2):  # 0->q, 1->k
                    bias = qb_sb if which == 0 else kb_sb
                    for f in range(KD):
                        ps = psum.tile([P, HTOK], f32, tag="big", bufs=2)
                        cb = which * D + P * f
                        for s2 in range(0, KD, 2):
                            nc.tensor.matmul(
                                ps, awqk_sb[:, s2:s2 + 2, cb:cb + P],
                                xn8[:, s2:s2 + 2, c0:c1],
                                start=(s2 == 0), stop=(s2 == KD - 2),
                                perf_mode=DR)
                        nc.scalar.activation(
                            qkloc[hh][:, which, f, :], ps, AF.Identity,
                            bias=bias[:, f:f + 1], scale=1.0 / SWA)
                for r in range(NCORES):
                    nc.sync.dma_start(
                        PAYQ[hh][r].rearrange("w p c -> p w c"),
                        qkloc[hh][:, :, r, :])
                for t in range(2):  # token sub-blocks of this half
                    tb = c0 + P * t
                    for f in range(KD):
                        psv = psum.tile([P, P], f32, tag="big", bufs=2)
                        for k in range(KD):
                            nc.tensor.matmul(
                                psv, xnT[:, k, tb:tb + P],
                                awv_sb[:, k, P * f:P * (f + 1)],
                                start=(k == 0), stop=(k == KD - 1))
                        nc.vector.tensor_tensor(
                            out=vloc[hh][:, t, f, :], in0=psv,
                            in1=vbc[:, f, :], op=OP.add)
                for r in range(NCORES):
                    nc.sync.dma_start(
                        PAYV[hh][r].rearrange("p (t c) -> p t c", t=2),
                        vloc[hh][:, :, r, :])

        def cast_half(hh):
            c0, c1 = HTOK * hh, HTOK * (hh + 1)
            for k in range(KD):  # bf16 -> fp8 cast for the q/k operand
                nc.gpsimd.tensor_copy(xn8[:, k, c0:c1], xnT[:, k, c0:c1])

        def qk_trigger(hh):
            with nc.named_scope("a2aq"):
                nc.gpsimd.collective_compute(
                    "AllToAll", OP.bypass, replica_groups=RG,
                    ins=[PAYQ[hh][:].opt()], outs=[PAYQO[hh][:].opt()])

        def v_trigger(hh):
            with nc.named_scope("a2aq"):
                nc.gpsimd.collective_compute(
                    "AllToAll", OP.bypass, replica_groups=RG,
                    ins=[PAYV[hh][:].opt()], outs=[PAYVO[hh][:].opt()])

        with nc.named_scope("ln1"):
            emit_ln(lambda k: xT_sb[:, k, :], l1g_sb, l1b_sb,
                    lambda k, c0, c1: xnT[:, k, c0:c1], "l1", 0)
        cast_half(0)
        qkv_half(0)
        with nc.named_scope("ln1"):
            emit_ln(lambda k: xT_sb[:, k, :], l1g_sb, l1b_sb,
                    lambda k, c0, c1: xnT[:, k, c0:c1], "l1", 1)
        cast_half(1)
        qk_trigger(0)
        v_trigger(0)
        qkv_half(1)
        qk_trigger(1)
        v_trigger(1)
        s_qkv.close()

        kT = ats.tile([DH // 2, 2, 2, NCORES, QCH], fp8)
        qT = ats.tile([DH // 2, 2, 2, NCORES, QCH], fp8)
        vA = ats.tile([P, 2, 2 * NCORES, HL, DH + 1], bf16)
        num_sb = ats.tile([P, 2, NCORES, QCH], bf16)
        den_sb = [ats.tile([97, 4, QCH], bf16, name=f"den{b}")
                  for b in range(2)]
        for b in range(2):
            nc.gpsimd.memset(den_sb[b], 1.0)
        nc.vector.memset(vA[:, :, :, :, DH:DH + 1], 1.0)

        def assemble(b):
            for r in range(NCORES):
                nc.sync.dma_start(
                    qT[:, :, :, r, :],
                    PAYQO[b][r, 0].rearrange(
                        "(h i pp) c -> pp h i c", h=2, i=2))
                nc.sync.dma_start(
                    kT[:, :, :, r, :],
                    PAYQO[b][r, 1].rearrange(
                        "(h i pp) c -> pp h i c", h=2, i=2))
            for r in range(NCORES):
                nc.sync.dma_start(
                    vA[:, b, 2 * r:2 * r + 2, :, 0:DH],
                    PAYVO[b][r].rearrange(
                        "p (t h d) -> p t h d", t=2, h=HL))

        def attn_batch(b, qcs=range(NCORES)):
            with nc.named_scope("attn"):
                for qc in qcs:
                    nkb = 2 * qc + 2
                    accs = [psum.tile([DH + 1, QCH], f32, tag="sa",
                                      bufs=2, name=f"ac{b}_{qc}_{h}")
                            for h in range(HL)]
                    groups = []
                    kb0 = 0
                    while kb0 < nkb:
                        gn = min(4, nkb - kb0)
                        groups.append((kb0, gn))
                        kb0 += gn
                    for (g0, gn) in groups:
                        ws = []
                        for h in range(HL):
                            sc = psum.tile([P, 4, QCH], f32, tag=f"sc{h}",
                                           bufs=1)
                            for j in range(gn):
                                kb = g0 + j
                                nc.tensor.matmul(
                                    sc[:, j, :],
                                    kT[:, h, :, kb // 2,
                                       P * (kb % 2):P * (kb % 2 + 1)],
                                    qT[:, h, :, qc, :],
                                    start=True, stop=True, perf_mode=DR,
                                    skip_group_check=True)
                            w = wp.tile([P, 4, QCH], bf16, tag=f"w{h}")
                            nc.scalar.activation(w[:, 0:gn, :], sc[:, 0:gn, :],
                                                 AF.Exp, scale=0.125)
                            if g0 + gn == nkb:  # diagonal pair: 0/1 mask
                                nc.gpsimd.tensor_tensor(
                                    out=w[:, gn - 2:gn, :],
                                    in0=w[:, gn - 2:gn, :],
                                    in1=mkb, op=OP.mult)
                            ws.append(w)
                        for h in range(HL):
                            for j in range(gn):
                                kb = g0 + j
                                nc.tensor.matmul(
                                    accs[h], vA[:, b, kb, h, :],
                                    ws[h][:, j, :],
                                    start=(kb == 0), stop=(kb == nkb - 1),
                                    skip_group_check=True)
                    for h in range(HL):
                        nc.vector.tensor_copy(
                            num_sb[DH * h:DH * (h + 1), b, qc, :],
                            accs[h][0:DH, :])
                        idx = 2 * qc + h
                        bb = 32 * (idx % 4)
                        nc.vector.tensor_copy(
                            den_sb[b][bb:bb + 1, idx // 4, :],
                            accs[h][DH:DH + 1, :])

        def norm_batch(b):
            with nc.named_scope("norm"):
                with nc.allow_low_precision(reason="f32r softmax denom"):
                    nc.vector.reciprocal(den_sb[b], den_sb[b])
                for qc in range(NCORES):
                    rb = psum.tile([P, QCH], f32, tag="big", bufs=2)
                    nc.tensor.matmul(rb, sel16[:, qc % 2, :],
                                     den_sb[b][:, qc // 2, :],
                                     start=True, stop=True)
                    nc.vector.tensor_tensor(out=num_sb[:, b, qc, :],
                                            in0=num_sb[:, b, qc, :],
                                            in1=rb, op=OP.mult)
                for r in range(NCORES):
                    nc.sync.dma_start(cc2_in[b][r], num_sb[:, b, r, :])
            with nc.named_scope("a2ao"):
                nc.gpsimd.collective_compute(
                    "AllToAll", OP.bypass, replica_groups=RG,
                    ins=[cc2_in[b][:].opt()], outs=[cc2_out[b][:].opt()])

        assemble(0)
        attn_batch(0)
        assemble(1)
        norm_batch(0)
        attn_batch(1, range(0, 4))

        # proj pools are ctx-level (batch 0 proj overlaps b1 attention)
        aF = [prs.tile([P, KD, QCH], bf16, name=f"aF{b}") for b in range(2)]
        for j in range(KD):
            nc.sync.dma_start(aF[0][:, j, :], cc2_out[0][j])

        def proj_batch(b):
            c0 = HTOK * b
            with nc.named_scope("proj"):
                if b == 1:
                    for j in range(KD):
                        nc.sync.dma_start(aF[1][:, j, :], cc2_out[1][j])
                for f in range(KD):
                    pwt = wgt.tile([P, KD, P], bf16, tag="pw", bufs=2)
                    nc.sync.dma_start(pwt, pw[f])
                    ps = psum.tile([P, HTOK], f32, tag="big", bufs=2)
                    for j in range(KD):
                        nc.tensor.matmul(ps, pwt[:, j, :], aF[b][:, j, :],
                                         start=(j == 0), stop=(j == KD - 1))
                    t1 = lnt.tile([P, HTOK], f32, tag="pj", bufs=2)
                    nc.vector.tensor_scalar_add(t1, ps, pb_sb[:, f:f + 1])
                    nc.gpsimd.tensor_tensor(
                        out=h1T[:, f, c0:c0 + HTOK], in0=t1,
                        in1=xT_sb[:, f, c0:c0 + HTOK], op=OP.add)

        def fc1_half(hh):
            c0, c1 = HTOK * hh, HTOK * (hh + 1)
            with nc.named_scope("fc1"):
                for j in range(KDI):
                    fwt = wgt.tile([P, KD, P], fp8 if FC1_DR else bf16,
                                   tag="fw", bufs=3)
                    nc.sync.dma_start(fwt, fw[j])
                    ps = psum.tile([P, HTOK], f32, tag="big", bufs=2)
                    if FC1_DR:
                        for s2 in range(0, KD, 2):
                            nc.tensor.matmul(
                                ps, fwt[:, s2:s2 + 2, :],
                                mT2[:, s2:s2 + 2, c0:c1],
                                start=(s2 == 0), stop=(s2 == KD - 2),
                                perf_mode=DR)
                    else:
                        for k in range(KD):
                            nc.tensor.matmul(
                                ps, fwt[:, k, :], mT2[:, k, c0:c1],
                                start=(k == 0), stop=(k == KD - 1))
                    nc.scalar.activation(hT[:, j, c0:c1], ps,
                                         AF.Gelu_apprx_tanh,
                                         bias=fb_sb[:, j:j + 1],
                                         scale=(1.0 / SWF) if FC1_DR
                                         else 1.0)

        proj_batch(0)
        with nc.named_scope("ln2"):
            emit_ln(lambda k: h1T[:, k, :], l2g_sb, l2b_sb,
                    lambda k, c0, c1: mT2[:, k, c0:c1], "l2", 0)
        attn_batch(1, range(4, NCORES))
        norm_batch(1)
        s_att.close()
        # mlp pools opened before LN2 emission (LIFO-safe: qkvs closed)
        mlp = ctx.enter_context(tc.tile_pool(name="mlp", bufs=1))
        hT = mlp.tile([P, KDI, TOK], bf16)

        fc1_half(0)
        proj_batch(1)
        with nc.named_scope("ln2"):
            emit_ln(lambda k: h1T[:, k, :], l2g_sb, l2b_sb,
                    lambda k, c0, c1: mT2[:, k, c0:c1], "l2", 1)
        fc1_half(1)

        def fc2_all():
            with nc.named_scope("fc2"):
                for f in range(KD):
                    gwts = []
                    for gg in range(2):
                        gwt = wgt.tile([P, KDI // 2, P], bf16, tag="gw",
                                       bufs=3)
                        nc.sync.dma_start(
                            gwt, gw[f, :, gg * KDI // 2:(gg + 1) * KDI // 2])
                        gwts.append(gwt)
                    for hh in range(2):
                        c0, c1 = HTOK * hh, HTOK * (hh + 1)
                        ps = psum.tile([P, HTOK], f32, tag="big", bufs=2)
                        for k in range(KDI):
                            nc.tensor.matmul(ps, gwts[k // 16][:, k % 16, :],
                                             hT[:, k, c0:c1],
                                             start=(k == 0),
                                             stop=(k == KDI - 1))
                        o = lnt.tile([P, HTOK], f32, tag="ot", bufs=2)
                        nc.vector.tensor_scalar_add(o, ps, gb_sb[:, f:f + 1])
                        nc.gpsimd.tensor_tensor(out=o, in0=o,
                                                in1=h1T[:, f, c0:c1],
                                                op=OP.add)
                        nc.sync.dma_start(outT[P * f:P * (f + 1), c0:c1], o)

        fc2_all()

    nc.compile()
    return nc


def shard_inputs(inputs):
    """Full inputs -> list of 8 per-core input dicts (host-side layout only)."""
    bf16 = ml_dtypes.bfloat16
    fp8 = ml_dtypes.float8_e4m3
    f32 = np.float32
    hs = np.asarray(inputs["hidden_states"], f32)          # [B, S, D]
    attn_w = np.asarray(inputs["attn_w"], f32)
    attn_b = np.ascontiguousarray(np.asarray(inputs["attn_b"], f32))

    def col(v):  # [D] -> [P, KD]
        return np.ascontiguousarray(np.asarray(v, f32).reshape(KD, P).T)

    aw3 = attn_w.reshape(KD, P, 3 * D).transpose(1, 0, 2)
    awqk = np.ascontiguousarray((aw3[:, :, 0:2 * D] * SWA).astype(fp8))
    awv = np.ascontiguousarray(aw3[:, :, 2 * D:3 * D].astype(bf16))
    pw = np.ascontiguousarray(np.asarray(inputs["proj_w"], f32)
                              .reshape(KD, P, KD, P).transpose(2, 1, 0, 3)
                              .astype(bf16))
    fdt, fsc = (fp8, SWF) if FC1_DR else (bf16, 1.0)
    fwv = np.ascontiguousarray((np.asarray(inputs["fc_w"], f32) * fsc)
                               .reshape(KD, P, KDI, P).transpose(2, 1, 0, 3)
                               .astype(fdt))
    gwv = np.ascontiguousarray(np.asarray(inputs["fc2_w"], f32)
                               .reshape(KDI, P, KD, P).transpose(2, 1, 0, 3)
                               .astype(bf16))
    pbv = col(inputs["proj_b"])
    fbv = np.ascontiguousarray(np.asarray(inputs["fc_b"], f32)
                               .reshape(KDI, P).T)
    gbv = col(inputs["fc2_b"])
    l1gv, l1bv = col(inputs["ln1_g"]), col(inputs["ln1_b"])
    l2gv, l2bv = col(inputs["ln2_g"]), col(inputs["ln2_b"])

    ii, jj = np.meshgrid(np.arange(P), np.arange(QCH), indexing="ij")
    mkv = np.where(np.stack([(jj >= ii), (jj >= ii + P)]), 1.0, 0.0)
    mkv = mkv.astype(f32)
    # sel[r, v, p] = 1 iff r == 32 * ((2q + p//64) % 4) with q%2 == v
    rr, vv, pp = np.meshgrid(np.arange(97), np.arange(2),
                             np.arange(P), indexing="ij")
    selv = (rr == 32 * ((2 * vv + pp // DH) % 4)).astype(f32)

    maps = []
    for c in range(NCORES):
        xT_c = np.ascontiguousarray(np.concatenate(
            [hs[0, QCH * c:QCH * (c + 1)], hs[1, QCH * c:QCH * (c + 1)]],
            axis=0).T)
        maps.append({
            "xT": xT_c, "awqk": awqk, "awv": awv, "ab": attn_b,
            "pw": pw, "pb": pbv, "fw": fwv, "fb": fbv, "gw": gwv, "gb": gbv,
            "l1g": l1gv, "l1b": l1bv, "l2g": l2gv, "l2b": l2bv, "mk": mkv,
            "sel": selv.astype(ml_dtypes.bfloat16),
        })
    return maps


def unshard(results):
    out = np.empty((B, S, D), np.float32)
    for c, r in enumerate(results):
        o = np.asarray(r["outT"])            # [D, TOK]
        out[0, QCH * c:QCH * (c + 1)] = o[:, 0:HTOK].T
        out[1, QCH * c:QCH * (c + 1)] = o[:, HTOK:TOK].T
    return out


def kernel(**inputs):
    global _CACHED_NC
    from concourse.bass_utils import run_bass_kernel_spmd
    if _CACHED_NC is None:
        _CACHED_NC = build_nc()
    in_maps = shard_inputs(inputs)
    res = run_bass_kernel_spmd(_CACHED_NC, in_maps,
                               core_ids=list(range(NCORES)))
    return unshard(res.results)
